# revision 11
# baseline (speedup 1.0000x reference)
"""Trainium2 Bass kernel for nn_Attention1D (channel-attention with LayerNorm).

Computation (per batch b):
    xn = LayerNorm_c(x) * gamma          (channel-wise LN over C=512)
    qkv = w_qkv @ xn                     (1x1 conv, [1536,512]@[512,N])
    per head: sim = (q*scale) @ k^T over N -> [64,64]; attn = softmax(sim)
    out = attn @ v -> [512, N]
    y = w_out @ out + b_out

Distribution: data-parallel over batch B=8 across the 8 NeuronCores.

Per-core layout strategy: channels on partitions, sequence on free dim.
  - LN stats (sum, sum-sq over C) via ones-vector matmuls (partition reduce).
  - mean/rstd broadcast along partitions via K=1 outer-product matmuls.
  - xn materialized once; q,k computed transposed ([n,o], route lhsT=xn) so the
    sim contraction over N runs directly on the tensor engine; v computed
    un-transposed ([o,n], route rhs=xn) so attn@v and w_out contraction work
    directly.  sim accumulated in PSUM across the whole N loop (2 heads packed
    per 128-wide PSUM tile; block-diagonal attn zeroes the cross terms).
  - v spilled to internal DRAM between pass 1 and pass 2 (SBUF is too small to
    hold v + weights + working set).
  - big matmuls run in float32r (1 PE cycle/row vs 4 for float32 at moving
    dim >= 256).  fp32r operands must be produced rounded, so matmul-feeding
    tiles are allocated as float32r and written by vector/scalar ops (which
    round on write).  The sim contraction stays float32 for precision.
"""

import os

import numpy as np

import concourse.bass as bass
import concourse.bacc as bacc
import concourse.tile as tile
from concourse import mybir
from concourse.masks import make_identity

F32 = mybir.dt.float32
AF = mybir.ActivationFunctionType
AX = mybir.AxisListType

B, C, N = 8, 512, 8192
HEADS, D = 8, 64
HID = HEADS * D  # 512
EPS = 1e-5
NB = N // 512  # 16 blocks of 512 seq positions
CCH = C // 128  # 4 channel chunks

# float32r for the big matmuls unless KMM_DT=f32 is set.
FR = mybir.dt.float32r if os.environ.get("KMM_DT", "f32r") == "f32r" else F32
# sim (q@kT over N) operand dtype: f32 (default, 4 cyc/col) or bf16 (1 cyc/col)
SIM_DT = mybir.dt.bfloat16 if os.environ.get("KSIM", "f32") == "bf16" else F32


def build_program():
    nc = bacc.Bacc("TRN2", target_bir_lowering=False, debug=False, num_devices=8)

    x_d = nc.dram_tensor("xb", [C, N], FR, kind="ExternalInput").ap()
    wq_d = nc.dram_tensor("wqkt", [128, CCH, 2 * HID], FR, kind="ExternalInput").ap()
    wo_d = nc.dram_tensor("woutt", [128, CCH, C], FR, kind="ExternalInput").ap()
    wgv_d = nc.dram_tensor("wgv", [128, CCH, C], FR, kind="ExternalInput").ap()
    bo_d = nc.dram_tensor("boutc", [128, CCH], F32, kind="ExternalInput").ap()
    y_d = nc.dram_tensor("y", [C, N], F32, kind="ExternalOutput").ap()
    xn_d = nc.dram_tensor("xntmp", [C, N], FR).ap()  # internal scratch

    with tile.TileContext(nc) as tc:
        with (
            tc.tile_pool(name="singles", bufs=1) as singles,
            tc.tile_pool(name="xrpool", bufs=3) as xrpool,
            tc.tile_pool(name="sqpool", bufs=2) as sqpool,
            tc.tile_pool(name="xcpool", bufs=2) as xcpool,
            tc.tile_pool(name="xnpool", bufs=2) as xnpool,
            tc.tile_pool(name="qkpool", bufs=3) as qkpool,
            tc.tile_pool(name="rows", bufs=10) as rows,
            tc.tile_pool(name="attn", bufs=1) as attnp,
            tc.tile_pool(name="xnl", bufs=3) as xnl,
            tc.tile_pool(name="outp", bufs=2) as outp,
            tc.tile_pool(name="simps", bufs=1, space="PSUM") as simps,
            tc.tile_pool(name="wps", bufs=4, space="PSUM") as wps,
        ):
            # ---- constants ----
            wq_r = singles.tile([128, CCH, 2 * HID], FR)
            nc.sync.dma_start(wq_r[:], wq_d[:])
            wgv_s = singles.tile([128, CCH, C], FR)
            nc.sync.dma_start(wgv_s[:], wgv_d[:])
            wo_r = singles.tile([128, CCH, C], FR)
            nc.sync.dma_start(wo_r[:], wo_d[:])
            bo_s = singles.tile([128, CCH], F32)
            nc.sync.dma_start(bo_s[:], bo_d[:])
            ones_f = singles.tile([128, 128], F32)
            nc.vector.memset(ones_f[:], 1.0)
            if FR is F32:
                ones_col, ones_row = ones_f[:, 0:1], ones_f[0:1, :]
            else:
                ones_col = singles.tile([128, 1], FR)
                nc.scalar.copy(ones_col[:], ones_f[:, 0:1])
                ones_row = singles.tile([1, 128], FR)
                nc.scalar.copy(ones_row[:], ones_f[0:1, :])
            eps_t = singles.tile([1, 1], F32)
            nc.vector.memset(eps_t[:], EPS)
            ident = singles.tile([128, 128], F32)
            make_identity(nc, ident[:])

            # sim accumulators: 4 groups of 2 heads -> [128, 128] each
            sim_ps = [simps.tile([128, 128], F32, name=f"sim_ps{g}")
                      for g in range(4)]

            vstore_insts = []

            # =============== PASS 1: LN + qkv + sim accumulation ===========
            for nb in range(NB):
                ns = slice(nb * 512, (nb + 1) * 512)
                x_r = xrpool.tile([128, CCH, 512], FR)
                for cc in range(CCH):
                    nc.sync.dma_start(x_r[:, cc, :], x_d[cc * 128:(cc + 1) * 128, ns])

                xsq = sqpool.tile([128, CCH, 512], FR)
                nc.vector.tensor_mul(xsq[:], x_r[:], x_r[:])

                # sum and sum-of-squares over channels (partition reduce)
                ps_s = wps.tile([1, 512], F32, tag="w")
                ps_q = wps.tile([1, 512], F32, tag="w")
                for cc in range(CCH):
                    nc.tensor.matmul(ps_s[:], ones_col[:], x_r[:, cc, :],
                                     start=(cc == 0), stop=(cc == CCH - 1))
                for cc in range(CCH):
                    nc.tensor.matmul(ps_q[:], ones_col[:], xsq[:, cc, :],
                                     start=(cc == 0), stop=(cc == CCH - 1))

                # row stats: mean, rstd  (on [1,512] rows)
                mean_row = rows.tile([1, 512], FR, tag="r")
                nc.scalar.activation(mean_row[:], ps_s[:], AF.Identity, scale=1.0 / C)
                exq_row = rows.tile([1, 512], F32, tag="r")
                nc.scalar.activation(exq_row[:], ps_q[:], AF.Identity, scale=1.0 / C)
                msq_row = rows.tile([1, 512], F32, tag="r")
                nc.vector.tensor_mul(msq_row[:], mean_row[:], mean_row[:])
                var_row = rows.tile([1, 512], F32, tag="r")
                nc.vector.tensor_sub(var_row[:], exq_row[:], msq_row[:])
                std_row = rows.tile([1, 512], F32, tag="r")
                nc.scalar.activation(std_row[:], var_row[:], AF.Sqrt, bias=eps_t[:])
                rstd_row = rows.tile([1, 512], FR, tag="r")
                with nc.allow_low_precision(reason="fp32r rounding of rstd"):
                    nc.vector.reciprocal(rstd_row[:], std_row[:])

                # broadcast mean/rstd along partitions (K=1 outer product)
                mean_bc = wps.tile([128, 512], F32, tag="w")
                nc.tensor.matmul(mean_bc[:], ones_row[:], mean_row[:],
                                 start=True, stop=True)
                rstd_bc = wps.tile([128, 512], F32, tag="w")
                nc.tensor.matmul(rstd_bc[:], ones_row[:], rstd_row[:],
                                 start=True, stop=True)

                # xn = (x - mean) * rstd     (gamma and q-scale folded into W)
                xc_t = xcpool.tile([128, CCH, 512], F32)
                xn_t = xnpool.tile([128, CCH, 512], FR)
                for cc in range(CCH):
                    nc.vector.tensor_sub(xc_t[:, cc, :], x_r[:, cc, :], mean_bc[:])
                for cc in range(CCH):
                    nc.vector.tensor_mul(xn_t[:, cc, :], xc_t[:, cc, :], rstd_bc[:])

                # q,k transposed: [n128, 512] tiles; sim accumulation
                for nt in range(4):
                    nsl = slice(nt * 128, (nt + 1) * 128)
                    q_ps = wps.tile([128, 512], F32, tag="w")
                    k_ps = wps.tile([128, 512], F32, tag="w")
                    for cc in range(CCH):
                        lhs = xn_t[:, cc, nsl]
                        nc.tensor.matmul(q_ps[:], lhs, wq_r[:, cc, 0:512],
                                         start=(cc == 0), stop=(cc == CCH - 1))
                        nc.tensor.matmul(k_ps[:], lhs, wq_r[:, cc, 512:1024],
                                         start=(cc == 0), stop=(cc == CCH - 1))
                    q_sb = qkpool.tile([128, 512], SIM_DT)
                    nc.scalar.copy(q_sb[:], q_ps[:])
                    k_sb = qkpool.tile([128, 512], SIM_DT)
                    nc.scalar.copy(k_sb[:], k_ps[:])
                    first = (nb == 0 and nt == 0)
                    last = (nb == NB - 1 and nt == 3)
                    for g in range(4):
                        gs = slice(g * 128, (g + 1) * 128)
                        nc.tensor.matmul(sim_ps[g][:], q_sb[:, gs], k_sb[:, gs],
                                         start=first, stop=last,
                                         skip_group_check=True)

                # spill xn to DRAM for pass 2
                st_insts = []
                for cc in range(CCH):
                    st = nc.sync.dma_start(xn_d[cc * 128:(cc + 1) * 128, ns],
                                           xn_t[:, cc, :])
                    st_insts.append(st)
                vstore_insts.append(st_insts)

            # =============== softmax on the 4 packed sim tiles =============
            at_r = attnp.tile([128, 4, 128], FR)
            for g in range(4):
                sim_sb = rows.tile([128, 128], F32, tag="r")
                nc.scalar.copy(sim_sb[:], sim_ps[g][:])
                at = rows.tile([128, 128], F32, tag="r")
                nc.vector.memset(at[:], 0.0)
                for h0 in (0, 64):
                    sl = slice(h0, h0 + 64)
                    negmx = rows.tile([64, 1], F32, tag="r")
                    nc.vector.reduce_max(negmx[:], sim_sb[sl, sl], axis=AX.X,
                                         negate=True)
                    ssum = rows.tile([64, 1], F32, tag="r")
                    nc.scalar.activation(at[sl, sl], sim_sb[sl, sl], AF.Exp,
                                         bias=negmx[:], accum_out=ssum[:])
                    rsum = rows.tile([64, 1], F32, tag="r")
                    nc.vector.reciprocal(rsum[:], ssum[:])
                    nc.vector.tensor_scalar_mul(at[sl, sl], at[sl, sl], rsum[:])
                nc.scalar.copy(at_r[:, g, :], at[:])

            # =============== fold M^T = (Wout @ A @ Wgv)^T on-chip =========
            # woaT[j,o] = sum_h A[h,j] WoutT[h,o]  (block-diag per group)
            woaT = attnp.tile([128, CCH, C], FR)
            for g in range(4):
                wps_t = wps.tile([128, 512], F32, tag="w")
                nc.tensor.matmul(wps_t[:], at_r[:, g, :], wo_r[:, g, :],
                                 start=True, stop=True)
                nc.scalar.copy(woaT[:, g, :], wps_t[:])
            # MT[c,o] = sum_j Wgv[j,c] woaT[j,o]
            mt_sb = attnp.tile([128, CCH, C], FR)
            for ck in range(CCH):
                mps = wps.tile([128, 512], F32, tag="w")
                for jc in range(CCH):
                    nc.tensor.matmul(mps[:], wgv_s[:, jc, ck * 128:(ck + 1) * 128],
                                     woaT[:, jc, :],
                                     start=(jc == 0), stop=(jc == CCH - 1))
                nc.scalar.copy(mt_sb[:, ck, :], mps[:])

            # =============== PASS 2: y = M @ xn + b_out ====================
            for nb in range(NB):
                ns = slice(nb * 512, (nb + 1) * 512)
                xn_l = xnl.tile([128, CCH, 512], FR)
                for cc in range(CCH):
                    ld = nc.sync.dma_start(xn_l[:, cc, :],
                                           xn_d[cc * 128:(cc + 1) * 128, ns])
                    tile.add_dep_helper(ld.ins, vstore_insts[nb][cc].ins, sync=True,
                                        reason="xntmp dram round-trip")
                for oc in range(CCH):
                    o_ps = wps.tile([128, 512], F32, tag="w")
                    for cc in range(CCH):
                        nc.tensor.matmul(o_ps[:], mt_sb[:, cc, oc * 128:(oc + 1) * 128],
                                         xn_l[:, cc, :],
                                         start=(cc == 0), stop=(cc == CCH - 1))
                    o_sb = outp.tile([128, 512], F32)
                    nc.scalar.activation(o_sb[:], o_ps[:], AF.Identity,
                                         bias=bo_s[:, oc:oc + 1])
                    nc.sync.dma_start(y_d[oc * 128:(oc + 1) * 128, ns], o_sb[:])

    nc.compile()
    return nc


_CACHED = {}


def _get_program():
    if "nc" not in _CACHED:
        _CACHED["nc"] = build_program()
    return _CACHED["nc"]


def make_in_maps(x, gamma, w_qkv, w_out, b_out):
    x = np.ascontiguousarray(np.asarray(x, dtype=np.float32))
    gamma = np.asarray(gamma, dtype=np.float32)
    w_qkv = np.asarray(w_qkv, dtype=np.float32)
    w_out = np.asarray(w_out, dtype=np.float32)
    b_out = np.asarray(b_out, dtype=np.float32)

    wg = w_qkv * gamma[None, :]
    wg[0:HID] *= D ** -0.5  # fold q scale
    wqkt = np.ascontiguousarray(
        wg[:2 * HID].T.reshape(CCH, 128, 2 * HID).transpose(1, 0, 2))
    wgv = np.ascontiguousarray(
        wg[2 * HID:].reshape(CCH, 128, C).transpose(1, 0, 2))
    woutt = np.ascontiguousarray(
        w_out.T.reshape(CCH, 128, C).transpose(1, 0, 2))
    boutc = np.ascontiguousarray(b_out.reshape(CCH, 128).T)

    return [
        {"xb": x[b], "wqkt": wqkt, "wgv": wgv, "woutt": woutt, "boutc": boutc}
        for b in range(B)
    ]


def kernel(x, gamma, w_qkv, w_out, b_out):
    from concourse.bass_utils import run_bass_kernel_spmd

    nc = _get_program()
    in_maps = make_in_maps(x, gamma, w_qkv, w_out, b_out)
    res = run_bass_kernel_spmd(nc, in_maps, list(range(B)))
    return np.stack([res.results[b]["y"] for b in range(B)], axis=0)


# revision 14
# speedup vs baseline: 1.9265x; 1.9265x over previous
"""Trainium2 Bass kernel for nn_Attention1D (channel-attention with LayerNorm).

Computation (per batch b):
    xn = LayerNorm_c(x) * gamma          (channel-wise LN over C=512)
    qkv = w_qkv @ xn                     (1x1 conv, [1536,512]@[512,N])
    per head: sim = (q*scale) @ k^T over N -> [64,64]; attn = softmax(sim)
    out = attn @ v -> [512, N]
    y = w_out @ out + b_out

Distribution: data-parallel over batch B=8 across the 8 NeuronCores.

Per-core strategy (channels on partitions, sequence on free dim):
  - LN stats (sum, sum-sq over C) via ones-vector matmuls (partition reduce);
    mean/rstd broadcast along partitions via K=1 outer-product matmuls.
  - q,k computed transposed ([n,o], lhsT=xn) so the sim contraction over N
    runs directly on the tensor engine; sim for all 8 heads accumulated in a
    single PSUM bank (4 groups of 2 heads side by side: the first matmul's
    start=True clears the whole bank, later groups' first matmuls overwrite
    where has_written is still clear, everything after accumulates).
  - The v projection, block-diagonal attention, and output projection fold
    algebraically into one matrix M = W_out @ A @ W_gv computed on-chip per
    batch (A = block-diag softmax).  Pass 2 is then just y = M @ xn + b.
  - xn spilled to internal DRAM between passes (SBUF can't hold it).
  - Pass 1 is software-pipelined with a 2-block skew (stats -> broadcasts ->
    qk/sim) so the tensor engine's in-order queue never waits on the LN
    stats chain.
  - Big matmuls run in float32r (1 PE cycle/row vs 4 for float32 at moving
    dim >= 256); fp32r operands are produced rounded by vector/scalar ops.
    The sim contraction stays float32 (KSIM=bf16 switches it to bf16).
"""

import os

import numpy as np

import concourse.bass as bass
import concourse.bacc as bacc
import concourse.tile as tile
from concourse import mybir

F32 = mybir.dt.float32
AF = mybir.ActivationFunctionType
AX = mybir.AxisListType

B, C, N = 8, 512, 8192
HEADS, D = 8, 64
HID = HEADS * D  # 512
EPS = 1e-5
NB = N // 512  # 16 blocks of 512 seq positions
CCH = C // 128  # 4 channel chunks

# float32r for the big matmuls unless KMM_DT=f32 is set.
FR = mybir.dt.float32r if os.environ.get("KMM_DT", "f32r") == "f32r" else F32
# sim (q@kT over N) operand dtype: f32 (default, 4 cyc/col) or bf16 (1 cyc/col)
SIM_DT = mybir.dt.bfloat16 if os.environ.get("KSIM", "bf16") == "bf16" else F32
# xn round-trip dtype: bf16 (default) halves pass-2 DMA traffic
XN_DT = mybir.dt.bfloat16 if os.environ.get("KXN", "bf16") == "bf16" else \
    mybir.dt.float32r


def build_program():
    nc = bacc.Bacc("TRN2", target_bir_lowering=False, debug=False, num_devices=8)

    x_d = nc.dram_tensor("xb", [C, N], FR, kind="ExternalInput").ap()
    wq_d = nc.dram_tensor("wqkt", [128, CCH, 2 * HID], FR, kind="ExternalInput").ap()
    wo_d = nc.dram_tensor("woutt", [128, CCH, C], FR, kind="ExternalInput").ap()
    wgv_d = nc.dram_tensor("wgv", [128, CCH, C], FR, kind="ExternalInput").ap()
    bo_d = nc.dram_tensor("boutc", [128, CCH], F32, kind="ExternalInput").ap()
    y_d = nc.dram_tensor("y", [C, N], F32, kind="ExternalOutput").ap()
    xn_d = nc.dram_tensor("xntmp", [C, N], XN_DT).ap()  # internal scratch

    # [C, ns] dram views reshaped to the sbuf tile layout [128, 2, 512]
    def dview(t_d, ns, half):
        return t_d[half * 256:(half + 1) * 256, ns].rearrange(
            "(k p) n -> p k n", p=128)

    with tile.TileContext(nc) as tc:
        with (
            tc.tile_pool(name="singles", bufs=1) as singles,
            tc.tile_pool(name="xrpool", bufs=3) as xrpool,
            tc.tile_pool(name="sqpool", bufs=2) as sqpool,
            tc.tile_pool(name="xcpool", bufs=2) as xcpool,
            tc.tile_pool(name="xnpool", bufs=2) as xnpool,
            tc.tile_pool(name="xnbpool", bufs=2) as xnbpool,
            tc.tile_pool(name="qkpool", bufs=3) as qkpool,
            tc.tile_pool(name="rows", bufs=10) as rows,
            tc.tile_pool(name="attn", bufs=1) as attnp,
            tc.tile_pool(name="xnl", bufs=3) as xnl,
            tc.tile_pool(name="outp", bufs=2) as outp,
            tc.tile_pool(name="simps", bufs=1, space="PSUM") as simps,
            tc.tile_pool(name="sbps", bufs=4, space="PSUM") as sbps,
            tc.tile_pool(name="wps", bufs=3, space="PSUM") as wps,
        ):
            # ---- constants ----
            wq_r = singles.tile([128, CCH, 2 * HID], FR)
            nc.sync.dma_start(wq_r[:], wq_d[:])
            wgv_s = singles.tile([128, CCH, C], FR)
            nc.sync.dma_start(wgv_s[:], wgv_d[:])
            wo_r = singles.tile([128, CCH, C], FR)
            nc.sync.dma_start(wo_r[:], wo_d[:])
            bo_s = singles.tile([128, CCH], F32)
            nc.sync.dma_start(bo_s[:], bo_d[:])
            ones_f = singles.tile([128, 128], F32)
            nc.vector.memset(ones_f[:], 1.0)
            if FR is F32:
                ones_col, ones_row = ones_f[:, 0:1], ones_f[0:1, :]
            else:
                ones_col = singles.tile([128, 1], FR)
                nc.scalar.copy(ones_col[:], ones_f[:, 0:1])
                ones_row = singles.tile([1, 128], FR)
                nc.scalar.copy(ones_row[:], ones_f[0:1, :])
            eps_t = singles.tile([1, 1], F32)
            nc.vector.memset(eps_t[:], EPS)

            # all 4 head-group sims in ONE psum bank [128, 4*128]
            sim_ps = simps.tile([128, 4, 128], F32)

            xn_store_insts = []

            # ======= PASS 1, software-pipelined (skew 2):
            #   stage A(nb):   x load, x^2, stats matmuls, row stats
            #   stage B(nb-1): mean/rstd broadcast, xn = (x-mean)*rstd
            #   stage C(nb-2): q/k matmuls, sim accumulation, xn spill
            stA, stB = {}, {}
            for it in range(NB + 2):
                a, b, c = it, it - 1, it - 2
                if a < NB:
                    ns = slice(a * 512, (a + 1) * 512)
                    x_r = xrpool.tile([128, CCH, 512], FR)
                    for h in range(2):
                        nc.gpsimd.dma_start(x_r[:, 2 * h:2 * h + 2, :],
                                            dview(x_d, ns, h))
                    xsq = sqpool.tile([128, CCH, 512], FR)
                    nc.gpsimd.tensor_mul(xsq[:], x_r[:], x_r[:])
                    ps_s = sbps.tile([1, 512], F32, tag="s")
                    ps_q = sbps.tile([1, 512], F32, tag="s")
                    for cc in range(CCH):
                        nc.tensor.matmul(ps_s[:], ones_col[:], x_r[:, cc, :],
                                         start=(cc == 0), stop=(cc == CCH - 1))
                    for cc in range(CCH):
                        nc.tensor.matmul(ps_q[:], ones_col[:], xsq[:, cc, :],
                                         start=(cc == 0), stop=(cc == CCH - 1))
                    mean_row = rows.tile([1, 512], FR, tag="r")
                    nc.scalar.activation(mean_row[:], ps_s[:], AF.Identity,
                                         scale=1.0 / C)
                    exq_row = rows.tile([1, 512], F32, tag="r")
                    nc.scalar.activation(exq_row[:], ps_q[:], AF.Identity,
                                         scale=1.0 / C)
                    msq_row = rows.tile([1, 512], F32, tag="r")
                    nc.vector.tensor_mul(msq_row[:], mean_row[:], mean_row[:])
                    var_row = rows.tile([1, 512], F32, tag="r")
                    nc.vector.tensor_sub(var_row[:], exq_row[:], msq_row[:])
                    std_row = rows.tile([1, 512], F32, tag="r")
                    nc.scalar.activation(std_row[:], var_row[:], AF.Sqrt,
                                         bias=eps_t[:])
                    rstd_row = rows.tile([1, 512], FR, tag="r")
                    with nc.allow_low_precision(reason="fp32r rounding of rstd"):
                        nc.vector.reciprocal(rstd_row[:], std_row[:])
                    stA[a] = (x_r, mean_row, rstd_row)

                if 0 <= b < NB:
                    x_r, mean_row, rstd_row = stA.pop(b)
                    mean_bc = sbps.tile([128, 512], F32, tag="s")
                    nc.tensor.matmul(mean_bc[:], ones_row[:], mean_row[:],
                                     start=True, stop=True)
                    rstd_bc = sbps.tile([128, 512], F32, tag="s")
                    nc.tensor.matmul(rstd_bc[:], ones_row[:], rstd_row[:],
                                     start=True, stop=True)
                    xc_t = xcpool.tile([128, CCH, 512], F32)
                    xn_t = xnpool.tile([128, CCH, 512], FR)
                    for cc in range(CCH):
                        nc.vector.tensor_sub(xc_t[:, cc, :], x_r[:, cc, :],
                                             mean_bc[:])
                    for cc in range(CCH):
                        nc.vector.tensor_mul(xn_t[:, cc, :], xc_t[:, cc, :],
                                             rstd_bc[:])
                    xn_b = xnbpool.tile([128, CCH, 512], XN_DT)
                    nc.vector.tensor_copy(xn_b[:], xn_t[:])
                    stB[b] = (xn_t, xn_b)

                if c >= 0:
                    ns = slice(c * 512, (c + 1) * 512)
                    xn_t, xn_b = stB.pop(c)
                    for nt in range(4):
                        nsl = slice(nt * 128, (nt + 1) * 128)
                        q_ps = wps.tile([128, 512], F32, tag="w")
                        k_ps = wps.tile([128, 512], F32, tag="w")
                        for cc in range(CCH):
                            lhs = xn_t[:, cc, nsl]
                            nc.tensor.matmul(q_ps[:], lhs, wq_r[:, cc, 0:512],
                                             start=(cc == 0), stop=(cc == CCH - 1))
                            nc.tensor.matmul(k_ps[:], lhs, wq_r[:, cc, 512:1024],
                                             start=(cc == 0), stop=(cc == CCH - 1))
                        q_sb = qkpool.tile([128, 512], SIM_DT)
                        nc.scalar.copy(q_sb[:], q_ps[:])
                        k_sb = qkpool.tile([128, 512], SIM_DT)
                        nc.scalar.copy(k_sb[:], k_ps[:])
                        first = (c == 0 and nt == 0)
                        last = (c == NB - 1 and nt == 3)
                        for g in range(4):
                            gs = slice(g * 128, (g + 1) * 128)
                            # only the very first matmul uses start=True: it
                            # clears the whole bank; other groups' first
                            # matmuls overwrite where has_written is clear.
                            # stop=True on each group's own last matmul.
                            nc.tensor.matmul(sim_ps[:, g, :], q_sb[:, gs],
                                             k_sb[:, gs],
                                             start=(first and g == 0),
                                             stop=last, skip_group_check=True)
                    st0 = []
                    for h in range(2):
                        st = nc.gpsimd.dma_start(dview(xn_d, ns, h),
                                                 xn_b[:, 2 * h:2 * h + 2, :])
                        st0.append(st)
                    xn_store_insts.append(st0)

            # =============== softmax on the 4 packed sim groups ============
            at_r = attnp.tile([128, 4, 128], FR)
            sim_sb = attnp.tile([128, 4, 128], F32)
            nc.scalar.copy(sim_sb[:], sim_ps[:])
            for g in range(4):
                at = rows.tile([128, 128], F32, tag="r")
                nc.vector.memset(at[:], 0.0)
                for h0 in (0, 64):
                    sl = slice(h0, h0 + 64)
                    negmx = rows.tile([64, 1], F32, tag="r")
                    nc.vector.reduce_max(negmx[:], sim_sb[sl, g, sl], axis=AX.X,
                                         negate=True)
                    ssum = rows.tile([64, 1], F32, tag="r")
                    nc.scalar.activation(at[sl, sl], sim_sb[sl, g, sl], AF.Exp,
                                         bias=negmx[:], accum_out=ssum[:])
                    rsum = rows.tile([64, 1], F32, tag="r")
                    nc.vector.reciprocal(rsum[:], ssum[:])
                    nc.vector.tensor_scalar_mul(at[sl, sl], at[sl, sl], rsum[:])
                nc.scalar.copy(at_r[:, g, :], at[:])

            # =============== fold M^T = (Wout @ A @ Wgv)^T on-chip =========
            # woaT[j,o] = sum_h A[h,j] WoutT[h,o]  (block-diag per group)
            woaT = attnp.tile([128, CCH, C], FR)
            for g in range(4):
                wps_t = wps.tile([128, 512], F32, tag="w")
                nc.tensor.matmul(wps_t[:], at_r[:, g, :], wo_r[:, g, :],
                                 start=True, stop=True)
                nc.scalar.copy(woaT[:, g, :], wps_t[:])
            # MT[c,o] = sum_j Wgv[j,c] woaT[j,o]  (bf16: pass-2 lhsT)
            mt_sb = attnp.tile([128, CCH, C], XN_DT)
            for ck in range(CCH):
                mps = wps.tile([128, 512], F32, tag="w")
                for jc in range(CCH):
                    nc.tensor.matmul(mps[:], wgv_s[:, jc, ck * 128:(ck + 1) * 128],
                                     woaT[:, jc, :],
                                     start=(jc == 0), stop=(jc == CCH - 1))
                nc.scalar.copy(mt_sb[:, ck, :], mps[:])

            # =============== PASS 2: y = M @ xn + b_out ====================
            for nb in range(NB):
                ns = slice(nb * 512, (nb + 1) * 512)
                xn_l = xnl.tile([128, CCH, 512], XN_DT)
                for h in range(2):
                    ld = nc.sync.dma_start(xn_l[:, 2 * h:2 * h + 2, :],
                                           dview(xn_d, ns, h))
                    tile.add_dep_helper(ld.ins, xn_store_insts[nb][h].ins,
                                        sync=True, reason="xntmp round-trip")
                y_sb = outp.tile([128, CCH, 512], F32)
                for oc in range(CCH):
                    o_ps = wps.tile([128, 512], F32, tag="w")
                    for cc in range(CCH):
                        nc.tensor.matmul(o_ps[:],
                                         mt_sb[:, cc, oc * 128:(oc + 1) * 128],
                                         xn_l[:, cc, :],
                                         start=(cc == 0), stop=(cc == CCH - 1))
                    nc.vector.tensor_scalar_add(y_sb[:, oc, :], o_ps[:],
                                                bo_s[:, oc:oc + 1])
                for h in range(2):
                    nc.gpsimd.dma_start(dview(y_d, ns, h),
                                        y_sb[:, 2 * h:2 * h + 2, :])

    nc.compile()
    return nc


_CACHED = {}


def _get_program():
    if "nc" not in _CACHED:
        _CACHED["nc"] = build_program()
    return _CACHED["nc"]


def make_in_maps(x, gamma, w_qkv, w_out, b_out):
    x = np.ascontiguousarray(np.asarray(x, dtype=np.float32))
    gamma = np.asarray(gamma, dtype=np.float32)
    w_qkv = np.asarray(w_qkv, dtype=np.float32)
    w_out = np.asarray(w_out, dtype=np.float32)
    b_out = np.asarray(b_out, dtype=np.float32)

    wg = w_qkv * gamma[None, :]
    wg[0:HID] *= D ** -0.5  # fold q scale
    wqkt = np.ascontiguousarray(
        wg[:2 * HID].T.reshape(CCH, 128, 2 * HID).transpose(1, 0, 2))
    wgv = np.ascontiguousarray(
        wg[2 * HID:].reshape(CCH, 128, C).transpose(1, 0, 2))
    woutt = np.ascontiguousarray(
        w_out.T.reshape(CCH, 128, C).transpose(1, 0, 2))
    boutc = np.ascontiguousarray(b_out.reshape(CCH, 128).T)

    return [
        {"xb": x[b], "wqkt": wqkt, "wgv": wgv, "woutt": woutt, "boutc": boutc}
        for b in range(B)
    ]


def kernel(x, gamma, w_qkv, w_out, b_out):
    from concourse.bass_utils import run_bass_kernel_spmd

    nc = _get_program()
    in_maps = make_in_maps(x, gamma, w_qkv, w_out, b_out)
    res = run_bass_kernel_spmd(nc, in_maps, list(range(B)))
    return np.stack([res.results[b]["y"] for b in range(B)], axis=0)


# revision 15
# speedup vs baseline: 3.4028x; 1.7663x over previous
"""Trainium2 Bass kernel for nn_Attention1D (channel-attention with LayerNorm).

Computation (per batch b):
    xn = LayerNorm_c(x) * gamma          (channel-wise LN over C=512)
    qkv = w_qkv @ xn                     (1x1 conv, [1536,512]@[512,N])
    per head: sim = (q*scale) @ k^T over N -> [64,64]; attn = softmax(sim)
    out = attn @ v -> [512, N]
    y = w_out @ out + b_out

Distribution: data-parallel over batch B=8 across the 8 NeuronCores.

Per-core strategy (channels on partitions, sequence on free dim):
  - LN stats (sum, sum-sq over C) via ones-vector matmuls (partition reduce);
    mean/rstd broadcast along partitions via K=1 outer-product matmuls.
  - q,k computed transposed ([n,o], lhsT=xn) so the sim contraction over N
    runs directly on the tensor engine; sim for all 8 heads accumulated in a
    single PSUM bank (4 groups of 2 heads side by side: the first matmul's
    start=True clears the whole bank, later groups' first matmuls overwrite
    where has_written is still clear, everything after accumulates).
  - The v projection, block-diagonal attention, and output projection fold
    algebraically into one matrix M = W_out @ A @ W_gv computed on-chip per
    batch (A = block-diag softmax).  Pass 2 is then just y = M @ xn + b.
  - xn spilled to internal DRAM between passes (SBUF can't hold it).
  - Pass 1 is software-pipelined with a 2-block skew (stats -> broadcasts ->
    qk/sim) so the tensor engine's in-order queue never waits on the LN
    stats chain.
  - Big matmuls run in float32r (1 PE cycle/row vs 4 for float32 at moving
    dim >= 256); fp32r operands are produced rounded by vector/scalar ops.
    The sim contraction stays float32 (KSIM=bf16 switches it to bf16).
"""

import os

import numpy as np

import concourse.bass as bass
import concourse.bacc as bacc
import concourse.tile as tile
from concourse import mybir

F32 = mybir.dt.float32
AF = mybir.ActivationFunctionType
AX = mybir.AxisListType

B, C, N = 8, 512, 8192
HEADS, D = 8, 64
HID = HEADS * D  # 512
EPS = 1e-5
NB = N // 512  # 16 blocks of 512 seq positions
CCH = C // 128  # 4 channel chunks

# float32r for the big matmuls unless KMM_DT=f32 is set.
FR = mybir.dt.float32r if os.environ.get("KMM_DT", "f32r") == "f32r" else F32
# sim (q@kT over N) operand dtype: f32 (default, 4 cyc/col) or bf16 (1 cyc/col)
SIM_DT = mybir.dt.bfloat16 if os.environ.get("KSIM", "bf16") == "bf16" else F32
# xn round-trip dtype: bf16 (default) halves pass-2 DMA traffic
XN_DT = mybir.dt.bfloat16 if os.environ.get("KXN", "f32r") == "bf16" else \
    mybir.dt.float32r


def build_program():
    nc = bacc.Bacc("TRN2", target_bir_lowering=False, debug=False, num_devices=8)

    x_d = nc.dram_tensor("xb", [C, N], FR, kind="ExternalInput").ap()
    wq_d = nc.dram_tensor("wqkt", [128, CCH, 2 * HID], FR, kind="ExternalInput").ap()
    wo_d = nc.dram_tensor("woutt", [128, CCH, C], FR, kind="ExternalInput").ap()
    wgv_d = nc.dram_tensor("wgv", [128, CCH, C], FR, kind="ExternalInput").ap()
    bo_d = nc.dram_tensor("boutc", [128, CCH], F32, kind="ExternalInput").ap()
    y_d = nc.dram_tensor("y", [C, N], F32, kind="ExternalOutput").ap()
    xn_d = nc.dram_tensor("xntmp", [C, N], XN_DT).ap()  # internal scratch

    # [C, ns] dram views reshaped to the sbuf tile layout [128, 2, 512]
    def dview(t_d, ns, half):
        return t_d[half * 256:(half + 1) * 256, ns].rearrange(
            "(k p) n -> p k n", p=128)

    with tile.TileContext(nc) as tc:
        with (
            tc.tile_pool(name="singles", bufs=1) as singles,
            tc.tile_pool(name="xrpool", bufs=3) as xrpool,
            tc.tile_pool(name="sqpool", bufs=2) as sqpool,
            tc.tile_pool(name="xcpool", bufs=2) as xcpool,
            tc.tile_pool(name="xnpool", bufs=2) as xnpool,
            tc.tile_pool(name="xnbpool", bufs=2) as xnbpool,
            tc.tile_pool(name="qkpool", bufs=3) as qkpool,
            tc.tile_pool(name="rows", bufs=10) as rows,
            tc.tile_pool(name="attn", bufs=1) as attnp,
            tc.tile_pool(name="xnl", bufs=3) as xnl,
            tc.tile_pool(name="outp", bufs=2) as outp,
            tc.tile_pool(name="simps", bufs=1, space="PSUM") as simps,
            tc.tile_pool(name="sbps", bufs=4, space="PSUM") as sbps,
            tc.tile_pool(name="wps", bufs=3, space="PSUM") as wps,
        ):
            # ---- constants ----
            wq_r = singles.tile([128, CCH, 2 * HID], FR)
            nc.sync.dma_start(wq_r[:], wq_d[:])
            wgv_s = singles.tile([128, CCH, C], FR)
            nc.sync.dma_start(wgv_s[:], wgv_d[:])
            wo_r = singles.tile([128, CCH, C], FR)
            nc.sync.dma_start(wo_r[:], wo_d[:])
            bo_s = singles.tile([128, CCH], F32)
            nc.sync.dma_start(bo_s[:], bo_d[:])
            ones_f = singles.tile([128, 128], F32)
            nc.vector.memset(ones_f[:], 1.0)
            if FR is F32:
                ones_col, ones_row = ones_f[:, 0:1], ones_f[0:1, :]
            else:
                ones_col = singles.tile([128, 1], FR)
                nc.scalar.copy(ones_col[:], ones_f[:, 0:1])
                ones_row = singles.tile([1, 128], FR)
                nc.scalar.copy(ones_row[:], ones_f[0:1, :])
            eps_t = singles.tile([1, 1], F32)
            nc.vector.memset(eps_t[:], EPS)

            # all 4 head-group sims in ONE psum bank [128, 4*128]
            sim_ps = simps.tile([128, 4, 128], F32)

            xn_store_insts = []

            # ======= PASS 1, software-pipelined (skew 2):
            #   stage A(nb):   x load, x^2, stats matmuls, row stats
            #   stage B(nb-1): mean/rstd broadcast, xn = (x-mean)*rstd
            #   stage C(nb-2): q/k matmuls, sim accumulation, xn spill
            stA, stB = {}, {}
            for it in range(NB + 2):
                a, b, c = it, it - 1, it - 2
                if a < NB:
                    ns = slice(a * 512, (a + 1) * 512)
                    x_r = xrpool.tile([128, CCH, 512], FR)
                    for h in range(2):
                        nc.gpsimd.dma_start(x_r[:, 2 * h:2 * h + 2, :],
                                            dview(x_d, ns, h))
                    xsq = sqpool.tile([128, CCH, 512], FR)
                    nc.gpsimd.tensor_mul(xsq[:], x_r[:], x_r[:])
                    ps_s = sbps.tile([1, 512], F32, tag="s")
                    ps_q = sbps.tile([1, 512], F32, tag="s")
                    for cc in range(CCH):
                        nc.tensor.matmul(ps_s[:], ones_col[:], x_r[:, cc, :],
                                         start=(cc == 0), stop=(cc == CCH - 1))
                    for cc in range(CCH):
                        nc.tensor.matmul(ps_q[:], ones_col[:], xsq[:, cc, :],
                                         start=(cc == 0), stop=(cc == CCH - 1))
                    mean_row = rows.tile([1, 512], FR, tag="r")
                    nc.scalar.activation(mean_row[:], ps_s[:], AF.Identity,
                                         scale=1.0 / C)
                    exq_row = rows.tile([1, 512], F32, tag="r")
                    nc.scalar.activation(exq_row[:], ps_q[:], AF.Identity,
                                         scale=1.0 / C)
                    msq_row = rows.tile([1, 512], F32, tag="r")
                    nc.vector.tensor_mul(msq_row[:], mean_row[:], mean_row[:])
                    var_row = rows.tile([1, 512], F32, tag="r")
                    nc.vector.tensor_sub(var_row[:], exq_row[:], msq_row[:])
                    std_row = rows.tile([1, 512], F32, tag="r")
                    nc.scalar.activation(std_row[:], var_row[:], AF.Sqrt,
                                         bias=eps_t[:])
                    rstd_row = rows.tile([1, 512], FR, tag="r")
                    with nc.allow_low_precision(reason="fp32r rounding of rstd"):
                        nc.vector.reciprocal(rstd_row[:], std_row[:])
                    stA[a] = (x_r, mean_row, rstd_row)

                if 0 <= b < NB:
                    x_r, mean_row, rstd_row = stA.pop(b)
                    mean_bc = sbps.tile([128, 512], F32, tag="s")
                    nc.tensor.matmul(mean_bc[:], ones_row[:], mean_row[:],
                                     start=True, stop=True)
                    rstd_bc = sbps.tile([128, 512], F32, tag="s")
                    nc.tensor.matmul(rstd_bc[:], ones_row[:], rstd_row[:],
                                     start=True, stop=True)
                    xc_t = xcpool.tile([128, CCH, 512], F32)
                    xn_t = xnpool.tile([128, CCH, 512], FR)
                    for cc in range(CCH):
                        nc.vector.tensor_sub(xc_t[:, cc, :], x_r[:, cc, :],
                                             mean_bc[:])
                    for cc in range(CCH):
                        nc.vector.tensor_mul(xn_t[:, cc, :], xc_t[:, cc, :],
                                             rstd_bc[:])
                    xn_b = xnbpool.tile([128, CCH, 512], XN_DT)
                    nc.vector.tensor_copy(xn_b[:], xn_t[:])
                    stB[b] = (xn_t, xn_b)

                if c >= 0:
                    ns = slice(c * 512, (c + 1) * 512)
                    xn_t, xn_b = stB.pop(c)
                    for nt in range(4):
                        nsl = slice(nt * 128, (nt + 1) * 128)
                        q_ps = wps.tile([128, 512], F32, tag="w")
                        k_ps = wps.tile([128, 512], F32, tag="w")
                        for cc in range(CCH):
                            lhs = xn_t[:, cc, nsl]
                            nc.tensor.matmul(q_ps[:], lhs, wq_r[:, cc, 0:512],
                                             start=(cc == 0), stop=(cc == CCH - 1))
                            nc.tensor.matmul(k_ps[:], lhs, wq_r[:, cc, 512:1024],
                                             start=(cc == 0), stop=(cc == CCH - 1))
                        q_sb = qkpool.tile([128, 512], SIM_DT)
                        nc.scalar.copy(q_sb[:], q_ps[:])
                        k_sb = qkpool.tile([128, 512], SIM_DT)
                        nc.scalar.copy(k_sb[:], k_ps[:])
                        first = (c == 0 and nt == 0)
                        last = (c == NB - 1 and nt == 3)
                        for g in range(4):
                            gs = slice(g * 128, (g + 1) * 128)
                            # only the very first matmul uses start=True: it
                            # clears the whole bank; other groups' first
                            # matmuls overwrite where has_written is clear.
                            # stop=True on each group's own last matmul.
                            nc.tensor.matmul(sim_ps[:, g, :], q_sb[:, gs],
                                             k_sb[:, gs],
                                             start=(first and g == 0),
                                             stop=last, skip_group_check=True)
                    st0 = []
                    for h in range(2):
                        st = nc.gpsimd.dma_start(dview(xn_d, ns, h),
                                                 xn_b[:, 2 * h:2 * h + 2, :])
                        st0.append(st)
                    xn_store_insts.append(st0)

            # =============== softmax on the 4 packed sim groups ============
            at_r = attnp.tile([128, 4, 128], FR)
            sim_sb = attnp.tile([128, 4, 128], F32)
            nc.scalar.copy(sim_sb[:], sim_ps[:])
            for g in range(4):
                at = rows.tile([128, 128], F32, tag="r")
                nc.vector.memset(at[:], 0.0)
                for h0 in (0, 64):
                    sl = slice(h0, h0 + 64)
                    negmx = rows.tile([64, 1], F32, tag="r")
                    nc.vector.reduce_max(negmx[:], sim_sb[sl, g, sl], axis=AX.X,
                                         negate=True)
                    ssum = rows.tile([64, 1], F32, tag="r")
                    nc.scalar.activation(at[sl, sl], sim_sb[sl, g, sl], AF.Exp,
                                         bias=negmx[:], accum_out=ssum[:])
                    rsum = rows.tile([64, 1], F32, tag="r")
                    nc.vector.reciprocal(rsum[:], ssum[:])
                    nc.vector.tensor_scalar_mul(at[sl, sl], at[sl, sl], rsum[:])
                nc.scalar.copy(at_r[:, g, :], at[:])

            # =============== fold M^T = (Wout @ A @ Wgv)^T on-chip =========
            # woaT[j,o] = sum_h A[h,j] WoutT[h,o]  (block-diag per group)
            woaT = attnp.tile([128, CCH, C], FR)
            for g in range(4):
                wps_t = wps.tile([128, 512], F32, tag="w")
                nc.tensor.matmul(wps_t[:], at_r[:, g, :], wo_r[:, g, :],
                                 start=True, stop=True)
                nc.scalar.copy(woaT[:, g, :], wps_t[:])
            # MT[c,o] = sum_j Wgv[j,c] woaT[j,o]  (bf16: pass-2 lhsT)
            mt_sb = attnp.tile([128, CCH, C], XN_DT)
            for ck in range(CCH):
                mps = wps.tile([128, 512], F32, tag="w")
                for jc in range(CCH):
                    nc.tensor.matmul(mps[:], wgv_s[:, jc, ck * 128:(ck + 1) * 128],
                                     woaT[:, jc, :],
                                     start=(jc == 0), stop=(jc == CCH - 1))
                nc.scalar.copy(mt_sb[:, ck, :], mps[:])

            # =============== PASS 2: y = M @ xn + b_out ====================
            for nb in range(NB):
                ns = slice(nb * 512, (nb + 1) * 512)
                xn_l = xnl.tile([128, CCH, 512], XN_DT)
                for h in range(2):
                    ld = nc.sync.dma_start(xn_l[:, 2 * h:2 * h + 2, :],
                                           dview(xn_d, ns, h))
                    tile.add_dep_helper(ld.ins, xn_store_insts[nb][h].ins,
                                        sync=True, reason="xntmp round-trip")
                y_sb = outp.tile([128, CCH, 512], F32)
                for oc in range(CCH):
                    o_ps = wps.tile([128, 512], F32, tag="w")
                    for cc in range(CCH):
                        nc.tensor.matmul(o_ps[:],
                                         mt_sb[:, cc, oc * 128:(oc + 1) * 128],
                                         xn_l[:, cc, :],
                                         start=(cc == 0), stop=(cc == CCH - 1))
                    nc.vector.tensor_scalar_add(y_sb[:, oc, :], o_ps[:],
                                                bo_s[:, oc:oc + 1])
                for h in range(2):
                    nc.gpsimd.dma_start(dview(y_d, ns, h),
                                        y_sb[:, 2 * h:2 * h + 2, :])

    nc.compile()
    return nc


_CACHED = {}


def _get_program():
    if "nc" not in _CACHED:
        _CACHED["nc"] = build_program()
    return _CACHED["nc"]


def make_in_maps(x, gamma, w_qkv, w_out, b_out):
    x = np.ascontiguousarray(np.asarray(x, dtype=np.float32))
    gamma = np.asarray(gamma, dtype=np.float32)
    w_qkv = np.asarray(w_qkv, dtype=np.float32)
    w_out = np.asarray(w_out, dtype=np.float32)
    b_out = np.asarray(b_out, dtype=np.float32)

    wg = w_qkv * gamma[None, :]
    wg[0:HID] *= D ** -0.5  # fold q scale
    wqkt = np.ascontiguousarray(
        wg[:2 * HID].T.reshape(CCH, 128, 2 * HID).transpose(1, 0, 2))
    wgv = np.ascontiguousarray(
        wg[2 * HID:].reshape(CCH, 128, C).transpose(1, 0, 2))
    woutt = np.ascontiguousarray(
        w_out.T.reshape(CCH, 128, C).transpose(1, 0, 2))
    boutc = np.ascontiguousarray(b_out.reshape(CCH, 128).T)

    return [
        {"xb": x[b], "wqkt": wqkt, "wgv": wgv, "woutt": woutt, "boutc": boutc}
        for b in range(B)
    ]


def kernel(x, gamma, w_qkv, w_out, b_out):
    from concourse.bass_utils import run_bass_kernel_spmd

    nc = _get_program()
    in_maps = make_in_maps(x, gamma, w_qkv, w_out, b_out)
    res = run_bass_kernel_spmd(nc, in_maps, list(range(B)))
    return np.stack([res.results[b]["y"] for b in range(B)], axis=0)


# revision 27
# speedup vs baseline: 219.1166x; 64.3931x over previous
"""Trainium2 Bass kernel for nn_Attention1D (channel-attention with LayerNorm).

Computation (per batch b):
    xn = LayerNorm_c(x) * gamma          (channel-wise LN over C=512)
    qkv = w_qkv @ xn                     (1x1 conv, [1536,512]@[512,N])
    per head: sim = (q*scale) @ k^T over N -> [64,64]; attn = softmax(sim)
    out = attn @ v -> [512, N]
    y = w_out @ out + b_out

Distribution: data-parallel over batch B=8 across the 8 NeuronCores.

Per-core strategy (channels on partitions, sequence on free dim):
  - LN stats (sum, sum-sq over C) via ones-vector matmuls (partition reduce);
    mean/rstd broadcast along partitions via K=1 outer-product matmuls.
  - q,k computed transposed ([n,o], lhsT=xn) so the sim contraction over N
    runs directly on the tensor engine; sim for all 8 heads accumulated in a
    single PSUM bank (4 groups of 2 heads side by side: the first matmul's
    start=True clears the whole bank, later groups' first matmuls overwrite
    where has_written is still clear, everything after accumulates).
  - The v projection, block-diagonal attention, and output projection fold
    algebraically into one matrix M = W_out @ A @ W_gv computed on-chip per
    batch (A = block-diag softmax).  Pass 2 is then just y = M @ xn + b.
  - xn spilled to internal DRAM between passes (SBUF can't hold it).
  - Pass 1 is software-pipelined with a 2-block skew (stats -> broadcasts ->
    qk/sim) so the tensor engine's in-order queue never waits on the LN
    stats chain.
  - Big matmuls run in float32r (1 PE cycle/row vs 4 for float32 at moving
    dim >= 256); fp32r operands are produced rounded by vector/scalar ops.
    The sim contraction stays float32 (KSIM=bf16 switches it to bf16).
"""

import os

import numpy as np

import concourse.bass as bass
import concourse.bacc as bacc
import concourse.tile as tile
from concourse import mybir

F32 = mybir.dt.float32
AF = mybir.ActivationFunctionType
AX = mybir.AxisListType

B, C, N = 8, 512, 8192
HEADS, D = 8, 64
HID = HEADS * D  # 512
EPS = 1e-5
NB = N // 512  # 16 blocks of 512 seq positions
CCH = C // 128  # 4 channel chunks

# float32r for the big matmuls unless KMM_DT=f32 is set.
FR = mybir.dt.float32r if os.environ.get("KMM_DT", "f32r") == "f32r" else F32
# sim (q@kT over N) operand dtype: f32 (default, 4 cyc/col) or bf16 (1 cyc/col)
SIM_DT = mybir.dt.bfloat16 if os.environ.get("KSIM", "bf16") == "bf16" else F32
# xn round-trip dtype: bf16 (default) halves pass-2 DMA traffic
XN_DT = mybir.dt.bfloat16 if os.environ.get("KXN", "f32r") == "bf16" else \
    mybir.dt.float32r


def build_program():
    nc = bacc.Bacc("TRN2", target_bir_lowering=False, debug=False, num_devices=8)

    x_d = nc.dram_tensor("xb", [C, N], FR, kind="ExternalInput").ap()
    wq_d = nc.dram_tensor("wqkt", [128, CCH, 2 * HID], FR, kind="ExternalInput").ap()
    wo_d = nc.dram_tensor("woutt", [128, CCH, C], FR, kind="ExternalInput").ap()
    wgv_d = nc.dram_tensor("wgv", [128, CCH, C], FR, kind="ExternalInput").ap()
    bo_d = nc.dram_tensor("boutc", [128, CCH], F32, kind="ExternalInput").ap()
    y_d = nc.dram_tensor("y", [C, N], F32, kind="ExternalOutput").ap()
    xn_d = nc.dram_tensor("xntmp", [C, N], XN_DT).ap()  # internal scratch

    # [C, ns] dram views reshaped to the sbuf tile layout [128, 2, 512]
    def dview(t_d, ns, half):
        return t_d[half * 256:(half + 1) * 256, ns].rearrange(
            "(k p) n -> p k n", p=128)

    with tile.TileContext(nc) as tc:
        with (
            tc.tile_pool(name="singles", bufs=1) as singles,
            tc.tile_pool(name="xrpool", bufs=3) as xrpool,
            tc.tile_pool(name="sqpool", bufs=2) as sqpool,
            tc.tile_pool(name="xcpool", bufs=2) as xcpool,
            tc.tile_pool(name="xnpool", bufs=2) as xnpool,
            tc.tile_pool(name="xnbpool", bufs=2) as xnbpool,
            tc.tile_pool(name="qkpool", bufs=3) as qkpool,
            tc.tile_pool(name="rows", bufs=9) as rows,
            tc.tile_pool(name="attn", bufs=1) as attnp,
            tc.tile_pool(name="xnl", bufs=3) as xnl,
            tc.tile_pool(name="outp", bufs=2) as outp,
            tc.tile_pool(name="simps", bufs=1, space="PSUM") as simps,
            tc.tile_pool(name="sbps", bufs=3, space="PSUM") as sbps,
            tc.tile_pool(name="wps", bufs=4, space="PSUM") as wps,
        ):
            # ---- constants ----
            wq_r = singles.tile([128, CCH, 2 * HID], FR)
            nc.sync.dma_start(wq_r[:], wq_d[:])
            wgv_s = singles.tile([128, CCH, C], FR)
            nc.sync.dma_start(wgv_s[:], wgv_d[:])
            wo_r = singles.tile([128, CCH, C], FR)
            nc.sync.dma_start(wo_r[:], wo_d[:])
            bo_s = singles.tile([128, CCH], F32)
            nc.sync.dma_start(bo_s[:], bo_d[:])
            ones_f = singles.tile([128, 128], F32)
            nc.vector.memset(ones_f[:], 1.0)
            if FR is F32:
                ones_col, ones_row = ones_f[:, 0:1], ones_f[0:1, :]
            else:
                ones_col = singles.tile([128, 1], FR)
                nc.scalar.copy(ones_col[:], ones_f[:, 0:1])
                ones_row = singles.tile([1, 128], FR)
                nc.scalar.copy(ones_row[:], ones_f[0:1, :])
            eps_t = singles.tile([1, 1], F32)
            nc.vector.memset(eps_t[:], EPS)

            # all 4 head-group sims in ONE psum bank [128, 4*128]
            sim_ps = simps.tile([128, 4, 128], F32)

            xn_store_insts = []

            # ======= PASS 1, software-pipelined (skew 2):
            #   stage A(nb):   x load, x^2, stats matmuls, row stats
            #   stage B(nb-1): mean/rstd broadcast, xn = (x-mean)*rstd
            #   stage C(nb-2): q/k matmuls, sim accumulation, xn spill
            stA, stB = {}, {}
            for it in range(NB + 2):
                a, b, c = it, it - 1, it - 2
                if a < NB:
                    ns = slice(a * 512, (a + 1) * 512)
                    x_r = xrpool.tile([128, CCH, 512], FR)
                    for h in range(2):
                        nc.gpsimd.dma_start(x_r[:, 2 * h:2 * h + 2, :],
                                            dview(x_d, ns, h))
                    xsq = sqpool.tile([128, CCH, 512], FR)
                    nc.gpsimd.tensor_mul(xsq[:], x_r[:], x_r[:])
                    ps_s = sbps.tile([1, 512], F32, tag="s")
                    ps_q = sbps.tile([1, 512], F32, tag="s")
                    for cc in range(CCH):
                        nc.tensor.matmul(ps_s[:], ones_col[:], x_r[:, cc, :],
                                         start=(cc == 0), stop=(cc == CCH - 1))
                    for cc in range(CCH):
                        nc.tensor.matmul(ps_q[:], ones_col[:], xsq[:, cc, :],
                                         start=(cc == 0), stop=(cc == CCH - 1))
                    mean_row = rows.tile([1, 512], FR, tag="r")
                    nc.scalar.activation(mean_row[:], ps_s[:], AF.Identity,
                                         scale=1.0 / C)
                    exq_row = rows.tile([1, 512], F32, tag="r")
                    nc.scalar.activation(exq_row[:], ps_q[:], AF.Identity,
                                         scale=1.0 / C)
                    msq_row = rows.tile([1, 512], F32, tag="r")
                    nc.vector.tensor_mul(msq_row[:], mean_row[:], mean_row[:])
                    var_row = rows.tile([1, 512], F32, tag="r")
                    nc.vector.tensor_sub(var_row[:], exq_row[:], msq_row[:])
                    std_row = rows.tile([1, 512], F32, tag="r")
                    nc.scalar.activation(std_row[:], var_row[:], AF.Sqrt,
                                         bias=eps_t[:])
                    rstd_row = rows.tile([1, 512], FR, tag="r")
                    with nc.allow_low_precision(reason="fp32r rounding of rstd"):
                        nc.vector.reciprocal(rstd_row[:], std_row[:])
                    stA[a] = (x_r, mean_row, rstd_row)

                if 0 <= b < NB:
                    x_r, mean_row, rstd_row = stA.pop(b)
                    mean_bc = sbps.tile([128, 512], F32, tag="s")
                    nc.tensor.matmul(mean_bc[:], ones_row[:], mean_row[:],
                                     start=True, stop=True)
                    rstd_bc = sbps.tile([128, 512], F32, tag="s")
                    nc.tensor.matmul(rstd_bc[:], ones_row[:], rstd_row[:],
                                     start=True, stop=True)
                    xc_t = xcpool.tile([128, CCH, 512], F32)
                    xn_t = xnpool.tile([128, CCH, 512], FR)
                    for cc in range(CCH):
                        nc.vector.tensor_sub(xc_t[:, cc, :], x_r[:, cc, :],
                                             mean_bc[:])
                    for cc in range(CCH):
                        nc.vector.tensor_mul(xn_t[:, cc, :], xc_t[:, cc, :],
                                             rstd_bc[:])
                    xn_b = xnbpool.tile([128, CCH, 512], XN_DT)
                    nc.vector.tensor_copy(xn_b[:], xn_t[:])
                    stB[b] = (xn_t, xn_b)

                if c >= 0:
                    ns = slice(c * 512, (c + 1) * 512)
                    xn_t, xn_b = stB.pop(c)
                    for nt in range(4):
                        nsl = slice(nt * 128, (nt + 1) * 128)
                        q_ps = wps.tile([128, 512], F32, tag="w")
                        k_ps = wps.tile([128, 512], F32, tag="w")
                        for cc in range(CCH):
                            lhs = xn_t[:, cc, nsl]
                            nc.tensor.matmul(q_ps[:], lhs, wq_r[:, cc, 0:512],
                                             start=(cc == 0), stop=(cc == CCH - 1))
                            nc.tensor.matmul(k_ps[:], lhs, wq_r[:, cc, 512:1024],
                                             start=(cc == 0), stop=(cc == CCH - 1))
                        q_sb = qkpool.tile([128, 512], SIM_DT)
                        nc.scalar.copy(q_sb[:], q_ps[:])
                        k_sb = qkpool.tile([128, 512], SIM_DT)
                        nc.scalar.copy(k_sb[:], k_ps[:])
                        first = (c == 0 and nt == 0)
                        last = (c == NB - 1 and nt == 3)
                        for g in range(4):
                            gs = slice(g * 128, (g + 1) * 128)
                            # only the very first matmul uses start=True: it
                            # clears the whole bank; other groups' first
                            # matmuls overwrite where has_written is clear.
                            # stop=True on each group's own last matmul.
                            nc.tensor.matmul(sim_ps[:, g, :], q_sb[:, gs],
                                             k_sb[:, gs],
                                             start=(first and g == 0),
                                             stop=last, skip_group_check=True)
                    st0 = []
                    for h in range(2):
                        st = nc.gpsimd.dma_start(dview(xn_d, ns, h),
                                                 xn_b[:, 2 * h:2 * h + 2, :])
                        st0.append(st)
                    xn_store_insts.append(st0)

            # ====== softmax on the 4 packed sim groups (phase-batched) =====
            at_r = attnp.tile([128, 4, 128], FR)
            sim_sb = attnp.tile([128, 4, 128], F32)
            nc.scalar.copy(sim_sb[:], sim_ps[:])
            ats = attnp.tile([128, 4, 128], F32)
            nc.vector.memset(ats[:], 0.0)
            ghs = [(g, h0) for g in range(4) for h0 in (0, 64)]
            negmx = rows.tile([64, 8], F32, tag="r")
            for i, (g, h0) in enumerate(ghs):
                sl = slice(h0, h0 + 64)
                nc.vector.reduce_max(negmx[0:64, i:i + 1], sim_sb[sl, g, sl],
                                     axis=AX.X, negate=True)
            ssum = rows.tile([64, 8], F32, tag="r")
            for i, (g, h0) in enumerate(ghs):
                sl = slice(h0, h0 + 64)
                nc.scalar.activation(ats[sl, g, sl], sim_sb[sl, g, sl], AF.Exp,
                                     bias=negmx[0:64, i:i + 1],
                                     accum_out=ssum[0:64, i:i + 1])
            rsum = rows.tile([64, 8], F32, tag="r")
            nc.vector.reciprocal(rsum[:], ssum[:])
            for i, (g, h0) in enumerate(ghs):
                sl = slice(h0, h0 + 64)
                nc.vector.tensor_scalar_mul(ats[sl, g, sl], ats[sl, g, sl],
                                            rsum[0:64, i:i + 1])
            nc.scalar.copy(at_r[:], ats[:])

            # =============== fold M^T = (Wout @ A @ Wgv)^T on-chip =========
            # woaT[j,o] = sum_h A[h,j] WoutT[h,o]  (block-diag per group)
            woaT = attnp.tile([128, CCH, C], FR)
            for g in range(4):
                wps_t = wps.tile([128, 512], F32, tag="w")
                nc.tensor.matmul(wps_t[:], at_r[:, g, :], wo_r[:, g, :],
                                 start=True, stop=True)
                nc.scalar.copy(woaT[:, g, :], wps_t[:])
            # MT[c,o] = sum_j Wgv[j,c] woaT[j,o]  (bf16: pass-2 lhsT)
            mt_sb = attnp.tile([128, CCH, C], XN_DT)
            for ck in range(CCH):
                mps = wps.tile([128, 512], F32, tag="w")
                for jc in range(CCH):
                    nc.tensor.matmul(mps[:], wgv_s[:, jc, ck * 128:(ck + 1) * 128],
                                     woaT[:, jc, :],
                                     start=(jc == 0), stop=(jc == CCH - 1))
                nc.scalar.copy(mt_sb[:, ck, :], mps[:])

            # =============== PASS 2: y = M @ xn + b_out ====================
            for nb in range(NB):
                ns = slice(nb * 512, (nb + 1) * 512)
                xn_l = xnl.tile([128, CCH, 512], XN_DT)
                for h in range(2):
                    ld = nc.sync.dma_start(xn_l[:, 2 * h:2 * h + 2, :],
                                           dview(xn_d, ns, h))
                    tile.add_dep_helper(ld.ins, xn_store_insts[nb][h].ins,
                                        sync=True, reason="xntmp round-trip")
                y_sb = outp.tile([128, CCH, 512], F32)
                for oc in range(CCH):
                    o_ps = wps.tile([128, 512], F32, tag="w")
                    for cc in range(CCH):
                        nc.tensor.matmul(o_ps[:],
                                         mt_sb[:, cc, oc * 128:(oc + 1) * 128],
                                         xn_l[:, cc, :],
                                         start=(cc == 0), stop=(cc == CCH - 1))
                    nc.vector.tensor_scalar_add(y_sb[:, oc, :], o_ps[:],
                                                bo_s[:, oc:oc + 1])
                for h in range(2):
                    nc.gpsimd.dma_start(dview(y_d, ns, h),
                                        y_sb[:, 2 * h:2 * h + 2, :])

    nc.compile()
    return nc


_CACHED = {}


def _get_program():
    if "nc" not in _CACHED:
        _CACHED["nc"] = build_program()
    return _CACHED["nc"]


def make_in_maps(x, gamma, w_qkv, w_out, b_out):
    x = np.ascontiguousarray(np.asarray(x, dtype=np.float32))
    gamma = np.asarray(gamma, dtype=np.float32)
    w_qkv = np.asarray(w_qkv, dtype=np.float32)
    w_out = np.asarray(w_out, dtype=np.float32)
    b_out = np.asarray(b_out, dtype=np.float32)

    wg = w_qkv * gamma[None, :]
    wg[0:HID] *= D ** -0.5  # fold q scale
    wqkt = np.ascontiguousarray(
        wg[:2 * HID].T.reshape(CCH, 128, 2 * HID).transpose(1, 0, 2))
    wgv = np.ascontiguousarray(
        wg[2 * HID:].reshape(CCH, 128, C).transpose(1, 0, 2))
    woutt = np.ascontiguousarray(
        w_out.T.reshape(CCH, 128, C).transpose(1, 0, 2))
    boutc = np.ascontiguousarray(b_out.reshape(CCH, 128).T)

    return [
        {"xb": x[b], "wqkt": wqkt, "wgv": wgv, "woutt": woutt, "boutc": boutc}
        for b in range(B)
    ]


def kernel(x, gamma, w_qkv, w_out, b_out):
    from concourse.bass_utils import run_bass_kernel_spmd

    nc = _get_program()
    in_maps = make_in_maps(x, gamma, w_qkv, w_out, b_out)
    res = run_bass_kernel_spmd(nc, in_maps, list(range(B)))
    return np.stack([res.results[b]["y"] for b in range(B)], axis=0)
